# revision 7
# baseline (speedup 1.0000x reference)
"""Trainium2 Bass kernel for nn_MultiScaleFusionLayer (moe_routing).

Mathematical structure of the reference (see problem statement):
  - Every stage (expert matmuls, gate logits, mixture combine, attention
    softmax) is position-wise along L.
  - The final output is `task` (computed from gates at l=0 and attention
    scores at l=0..3) broadcast along L=100.
  => Only input positions l in {0,1,2,3} ever influence the output.

Strategy:
  - Host slices inputs to l<4 and shards batch B=2048 across 8 cores
    (256 rows/core, 4 positions => N=1024 "rows" per core, l-major).
  - Activations are shipped feature-on-partition (FT layout) with a ones
    row appended, so every bias folds into the matmuls and matmul lhsT
    (stationary operand) never needs on-device transposition.
  - One fused weight matrix computes, per 128-row tile: shared experts E
    (256 cols), the id/img/txt-feature parts of all task experts
    (3 x 256 cols), and all 16 gate logits in a single pair of
    K-accumulated matmuls. The gate_share-dependent part of the task
    experts is added with one PE transpose + 3 accumulating matmuls.
  - Expert mixture (per-row gating) runs on DVE with broadcast access
    patterns; attention runs as 2 PE transposes + matmul + softmax
    (no max subtraction needed: logits are tiny).
  - Device returns task [256, 64] per core; host broadcasts to L=100.

Weight/psum column layout uses d-major expert blocks (col = d*4 + e) so
the e-reduction is an innermost-axis DVE reduce, and combined/att use
col = d*4 + g so the final task combine is a flat elementwise mul+reduce.
"""

import sys

if "/opt/trn_rl_repo" not in sys.path:
    sys.path.insert(0, "/opt/trn_rl_repo")

import numpy as np

B, L, D = 2048, 100, 64
OUT_L = 100
NCORES = 8
BC = B // NCORES          # 256 batch rows per core
LK = 4                    # positions that matter
N = BC * LK               # 1024 rows per core (n = l*BC + b)
NT = N // 128             # 8 row-tiles of 128 rows
KA, KB = 128, 65          # xin partition split (192 features + ones)
WC = 1040                 # fused weight cols: E 256 | T1f 256 | T2f 256 | T3f 256 | sels 16

_f32 = np.float32


def pack_weights(Ws, bs, Wt1, bt1, Wt2, bt2, Wt3, bt3,
                 Wgs, bgs, Wg1, bg1, Wg2, bg2, Wg3, bg3, Watt, batt):
    """Build the fused device weight arrays (all float32, replicated per core)."""
    def blk(Wt):  # [E,Din,D] -> [Din, D*4] with col = d*4+e
        return np.ascontiguousarray(np.transpose(Wt, (1, 2, 0)).reshape(Wt.shape[1], -1))

    def bblk(bt):  # [E,D] -> [D*4]
        return np.ascontiguousarray(bt.T.reshape(-1))

    Wbig = np.zeros((193, WC), dtype=_f32)
    Wbig[0:192, 0:256] = blk(Ws)
    Wbig[192, 0:256] = bblk(bs)
    Wbig[0:64, 256:512] = blk(Wt1)
    Wbig[192, 256:512] = bblk(bt1)
    Wbig[64:128, 512:768] = blk(Wt2)
    Wbig[192, 512:768] = bblk(bt2)
    Wbig[128:192, 768:1024] = blk(Wt3)
    Wbig[192, 768:1024] = bblk(bt3)
    Wbig[0:192, 1024:1028] = Wgs
    Wbig[192, 1024:1028] = bgs
    Wbig[0:64, 1028:1032] = Wg1
    Wbig[192, 1028:1032] = bg1
    Wbig[64:128, 1032:1036] = Wg2
    Wbig[192, 1032:1036] = bg2
    Wbig[128:192, 1036:1040] = Wg3
    Wbig[192, 1036:1040] = bg3

    wgs2 = np.concatenate([blk(Wt1), blk(Wt2), blk(Wt3)], axis=1)  # [64, 768]

    # Watt rows permuted to match combined's col = d*4+g layout.
    wap = np.ascontiguousarray(
        Watt.reshape(4, 64, 64).transpose(1, 0, 2).reshape(256, 64))
    watt2 = np.concatenate([wap[0:128], wap[128:256]], axis=1)  # [128, 128]

    battb = np.ascontiguousarray(np.tile(batt[None, :], (128, 1)))  # [128, 64]
    ident = np.eye(128, dtype=_f32)

    return {
        "wbig_a": np.ascontiguousarray(Wbig[0:128]),
        "wbig_b": np.ascontiguousarray(Wbig[128:193]),
        "wgs2": np.ascontiguousarray(wgs2.astype(_f32)),
        "watt2": np.ascontiguousarray(watt2.astype(_f32)),
        "battb": battb.astype(_f32),
        "ident": ident,
    }


def pack_core_inputs(id_feat, img_feat, txt_feat, core):
    """FT-layout per-core activations: xin_a [128, N], xin_b [65, N]."""
    sl = slice(core * BC, (core + 1) * BC)

    def ft(x):  # [BC, LK, D] -> [D, N] with n = l*BC + b
        return np.ascontiguousarray(x[sl, 0:LK, :].transpose(2, 1, 0).reshape(D, N))

    fid, fimg, ftxt = ft(id_feat), ft(img_feat), ft(txt_feat)
    xin_a = np.concatenate([fid, fimg], axis=0)
    xin_b = np.concatenate([ftxt, np.ones((1, N), dtype=_f32)], axis=0)
    return xin_a, xin_b


def numpy_forward(xin_a, xin_b, w):
    """Pure-numpy mirror of the device kernel (for validating the packing)."""
    xin = np.concatenate([xin_a, xin_b], axis=0)          # [193, N]
    Wbig = np.concatenate([w["wbig_a"], w["wbig_b"]], axis=0)
    P = xin.T @ Wbig                                      # [N, 1040]
    sel = P[:, 1024:1040]                                 # [N, 16]
    task = np.zeros((BC, D), dtype=_f32)
    att_all = np.zeros((2, 128, D, LK), dtype=_f32)
    comb_keep = np.zeros((2, 128, 256), dtype=_f32)
    wap = np.concatenate([w["watt2"][:, 0:64], w["watt2"][:, 64:128]], axis=0)
    for t in range(NT):
        rows = slice(t * 128, (t + 1) * 128)
        l, j = t // 2, t % 2
        E = P[rows, 0:256].reshape(128, D, 4)
        gs = (E * sel[rows, 0:4].reshape(128, 1, 4)).sum(-1)      # [128, 64]
        Tg = P[rows, 256:1024].reshape(128, 3, D, 4).copy()
        Tg += (gs @ w["wgs2"]).reshape(128, 3, D, 4)
        combined = np.zeros((128, D, 4), dtype=_f32)
        for g in range(3):
            combined[:, :, g] = (
                Tg[:, g] * sel[rows, 4 * (g + 1):4 * (g + 2)].reshape(128, 1, 4)
            ).sum(-1)
        combined[:, :, 3] = gs
        cflat = combined.reshape(128, 256)
        logits = cflat @ wap + w["battb"][0]
        ex = np.exp(logits)
        att = ex / ex.sum(-1, keepdims=True)
        att_all[j, :, :, l] = att
        if l == 0:
            comb_keep[j] = cflat
    for j in range(2):
        prod = comb_keep[j] * att_all[j].reshape(128, 256)
        task[j * 128:(j + 1) * 128] = prod.reshape(128, D, 4).sum(-1)
    return task


# ---------------------------------------------------------------------------
# Bass program
# ---------------------------------------------------------------------------

def build_program(mm_dtype="f32r"):
    """Build the Bass/Tile program. Returns (nc, out_name)."""
    import concourse.bass as bass
    import concourse.bacc as bacc
    import concourse.mybir as mybir
    import concourse.tile as tile
    from contextlib import ExitStack

    f32 = mybir.dt.float32
    mmdt = {
        "f32": mybir.dt.float32,
        "f32r": mybir.dt.float32r,
    }[mm_dtype]

    nc = bacc.Bacc("TRN2", target_bir_lowering=False, debug=False)

    xin_a = nc.dram_tensor("xin_a", [KA, N], f32, kind="ExternalInput").ap()
    xin_b = nc.dram_tensor("xin_b", [KB, N], f32, kind="ExternalInput").ap()
    wbig_a = nc.dram_tensor("wbig_a", [KA, WC], f32, kind="ExternalInput").ap()
    wbig_b = nc.dram_tensor("wbig_b", [KB, WC], f32, kind="ExternalInput").ap()
    wgs2 = nc.dram_tensor("wgs2", [64, 768], f32, kind="ExternalInput").ap()
    watt2 = nc.dram_tensor("watt2", [128, 128], f32, kind="ExternalInput").ap()
    battb = nc.dram_tensor("battb", [128, 64], f32, kind="ExternalInput").ap()
    ident = nc.dram_tensor("ident", [128, 128], f32, kind="ExternalInput").ap()
    out = nc.dram_tensor("task", [BC, D], f32, kind="ExternalOutput").ap()

    def mm(x):
        return x.bitcast(mmdt) if mmdt is not f32 else x

    Exp = mybir.ActivationFunctionType.Exp
    mult = mybir.AluOpType.mult
    add_op = mybir.AluOpType.add
    AX = mybir.AxisListType.X
    PSUM = bass.MemorySpace.PSUM

    with tile.TileContext(nc) as tc, ExitStack() as ctx:
        wp = ctx.enter_context(tc.tile_pool(name="w", bufs=1))
        xp = ctx.enter_context(tc.tile_pool(name="x", bufs=1))
        work = ctx.enter_context(tc.tile_pool(name="work", bufs=3))
        keep = ctx.enter_context(tc.tile_pool(name="keep", bufs=1))
        pbe_pool = ctx.enter_context(tc.tile_pool(name="pbe", bufs=1, space=PSUM))
        pbt_pool = ctx.enter_context(tc.tile_pool(name="pbt", bufs=2, space=PSUM))
        ps_small = ctx.enter_context(tc.tile_pool(name="pssm", bufs=1, space=PSUM))

        # --- load weights + activations (weights first: they gate the MMs)
        wa = wp.tile([KA, WC], f32, tag="wa")
        wb = wp.tile([KB, WC], f32, tag="wb")
        wg = wp.tile([64, 768], f32, tag="wg")
        wt = wp.tile([128, 128], f32, tag="wt")
        bb = wp.tile([128, 64], f32, tag="bb")
        idn = wp.tile([128, 128], f32, tag="idn")
        for dst, src in ((wa, wbig_a), (wb, wbig_b), (wg, wgs2),
                         (wt, watt2), (bb, battb), (idn, ident)):
            nc.sync.dma_start(dst[:], src[:])

        xa = xp.tile([KA, N], f32, tag="xa")
        xb = xp.tile([KB, N], f32, tag="xb")
        nc.sync.dma_start(xa[:], xin_a[:])
        nc.sync.dma_start(xb[:], xin_b[:])

        # persistent accumulators for the final combine
        att_all = [keep.tile([128, D, LK], f32, tag=f"att{j}", name=f"att{j}")
                   for j in range(2)]
        comb_keep = [keep.tile([128, 256], f32, tag=f"ck{j}", name=f"ck{j}")
                     for j in range(2)]

        for t in range(NT):
            l, j = t // 2, t % 2
            cols = bass.ts(t, 128)

            # pb_e: one PSUM bank holding E [0:256] + gate logits [256:272]
            # pb_t: two banks: T1f [0:256] T2f [256:512] | T3f [512:768]
            # pbca: one bank: combined^T [0:256] + att logits [256:320]
            # One matmul accumulation group per bank: exactly one start=True
            # (it lazily zeroes the whole 2 KiB bank) and one stop=True.
            pb_e = pbe_pool.tile([128, 272], f32, tag="pbe")
            pb_t = pbt_pool.tile([128, 768], f32, tag="pbt")
            gst_ps = ps_small.tile([64, 128], f32, tag="gst")
            ct_ps = ps_small.tile([128, 256], f32, tag="ctps")
            att_ps = ps_small.tile([128, 64], f32, tag="attps")

            nc.tensor.matmul(pb_e[:, 0:256], mm(xa[:, cols]),
                             mm(wa[:, 0:256]), start=True, stop=False)
            nc.tensor.matmul(pb_e[:, 256:272], mm(xa[:, cols]),
                             mm(wa[:, 1024:WC]), start=False, stop=False)
            nc.tensor.matmul(pb_e[:, 0:256], mm(xb[:, cols]),
                             mm(wb[:, 0:256]), start=False, stop=False)
            nc.tensor.matmul(pb_e[:, 256:272], mm(xb[:, cols]),
                             mm(wb[:, 1024:WC]), start=False, stop=True)

            nc.tensor.matmul(pb_t[:, 0:256], mm(xa[:, cols]),
                             mm(wa[:, 256:512]), start=True, stop=False)
            nc.tensor.matmul(pb_t[:, 256:512], mm(xa[:, cols]),
                             mm(wa[:, 512:768]), start=False, stop=False)
            nc.tensor.matmul(pb_t[:, 512:768], mm(xa[:, cols]),
                             mm(wa[:, 768:1024]), start=True, stop=False)
            nc.tensor.matmul(pb_t[:, 0:256], mm(xb[:, cols]),
                             mm(wb[:, 256:512]), start=False, stop=False)
            nc.tensor.matmul(pb_t[:, 256:512], mm(xb[:, cols]),
                             mm(wb[:, 512:768]), start=False, stop=False)
            nc.tensor.matmul(pb_t[:, 512:768], mm(xb[:, cols]),
                             mm(wb[:, 768:1024]), start=False, stop=False)

            sel = work.tile([128, 16], f32, tag="sel")
            nc.scalar.copy(sel[:], pb_e[:, 256:272])

            # gate_share = sum_e E[:, d, e] * sel_s[:, e]
            prod_s = work.tile([128, D, 4], f32, tag="prods")
            gs = work.tile([128, D], f32, tag="gs")
            sel_s_b = sel[:, 0:4].unsqueeze(1).broadcast_to((128, D, 4))
            nc.vector.tensor_tensor(prod_s[:], pb_e[:, 0:256].rearrange(
                "p (d e) -> p d e", e=4), sel_s_b, op=mult)
            nc.vector.reduce_sum(gs[:], prod_s[:], axis=AX)

            combined = comb_keep[j] if l == 0 else work.tile(
                [128, 256], f32, tag="comb")
            nc.vector.tensor_copy(combined[:, 3:256:4], gs[:])

            # transpose gate_share, add its expert contribution on PE
            nc.tensor.transpose(gst_ps[:], gs[:], idn[:])
            gst = work.tile([64, 128], f32, tag="gstsb")
            nc.scalar.copy(gst[:], gst_ps[:])
            nc.tensor.matmul(pb_t[:, 0:256], mm(gst[:]), mm(wg[:, 0:256]),
                             start=False, stop=False)
            nc.tensor.matmul(pb_t[:, 256:512], mm(gst[:]), mm(wg[:, 256:512]),
                             start=False, stop=True)
            nc.tensor.matmul(pb_t[:, 512:768], mm(gst[:]), mm(wg[:, 512:768]),
                             start=False, stop=True)

            # task gates: g_k = sum_e T_k[:, d, e] * sel_k[:, e]
            for g in range(3):
                prod_t = work.tile([128, D, 4], f32, tag=f"prodt{g}",
                                   name=f"prodt{g}")
                sel_b = sel[:, 4 * (g + 1):4 * (g + 2)].unsqueeze(
                    1).broadcast_to((128, D, 4))
                nc.vector.tensor_tensor(
                    prod_t[:], pb_t[:, 256 * g:256 * (g + 1)].rearrange(
                        "p (d e) -> p d e", e=4), sel_b, op=mult)
                nc.vector.reduce_sum(combined[:, g:256:4], prod_t[:], axis=AX)

            # attention: transpose combined, matmul with Watt, softmax
            nc.tensor.matmul(ct_ps[:, 0:128], combined[:, 0:128], idn[:],
                             is_transpose=True, start=True, stop=False)
            nc.tensor.matmul(ct_ps[:, 128:256], combined[:, 128:256], idn[:],
                             is_transpose=True, start=False, stop=True)
            ct = work.tile([128, 256], f32, tag="ct")
            nc.scalar.copy(ct[:, 0:128], ct_ps[:, 0:128])
            nc.scalar.copy(ct[:, 128:256], ct_ps[:, 128:256])
            nc.tensor.matmul(att_ps[:], mm(ct[:, 0:128]),
                             mm(wt[:, 0:64]), start=True, stop=False)
            nc.tensor.matmul(att_ps[:], mm(ct[:, 128:256]),
                             mm(wt[:, 64:128]), start=False, stop=True)

            logit = work.tile([128, D], f32, tag="logit")
            nc.vector.tensor_tensor(logit[:], att_ps[:], bb[:],
                                    op=add_op)
            ex = work.tile([128, D], f32, tag="ex")
            sumex = work.tile([128, 1], f32, tag="sumex")
            nc.scalar.activation(ex[:], logit[:], Exp, accum_out=sumex[:])
            rinv = work.tile([128, 1], f32, tag="rinv")
            nc.vector.reciprocal(rinv[:], sumex[:])
            nc.vector.tensor_scalar_mul(
                att_all[j][:, :, l], ex[:], rinv[:])

        # final combine + store
        for j in range(2):
            prod_f = work.tile([128, D, 4], f32, tag="prodf")
            task = work.tile([128, D], f32, tag="task")
            nc.vector.tensor_tensor(
                prod_f[:], comb_keep[j][:].rearrange("p (d g) -> p d g", g=4),
                att_all[j][:], op=mult)
            nc.vector.reduce_sum(task[:], prod_f[:], axis=AX)
            nc.sync.dma_start(out[j * 128:(j + 1) * 128, :], task[:])

    nc.compile()
    return nc, "task"


_PROGRAM_CACHE = {}


def _get_program(mm_dtype):
    if mm_dtype not in _PROGRAM_CACHE:
        _PROGRAM_CACHE[mm_dtype] = build_program(mm_dtype)
    return _PROGRAM_CACHE[mm_dtype]


def run_on_device(inputs, mm_dtype="f32r", trace=False):
    """Shard, run on 8 cores, return (task_full [B, D], BassKernelResults)."""
    from concourse.bass_utils import run_bass_kernel_spmd

    nc, out_name = _get_program(mm_dtype)
    w = pack_weights(**{k: inputs[k] for k in (
        "Ws", "bs", "Wt1", "bt1", "Wt2", "bt2", "Wt3", "bt3",
        "Wgs", "bgs", "Wg1", "bg1", "Wg2", "bg2", "Wg3", "bg3",
        "Watt", "batt")})
    in_maps = []
    for c in range(NCORES):
        xin_a, xin_b = pack_core_inputs(
            inputs["id_feat"], inputs["img_feat"], inputs["txt_feat"], c)
        in_maps.append({"xin_a": xin_a, "xin_b": xin_b, **w})
    res = run_bass_kernel_spmd(nc, in_maps, core_ids=list(range(NCORES)),
                               trace=trace)
    task_full = np.concatenate(
        [res.results[c][out_name] for c in range(NCORES)], axis=0)
    return task_full, res


def kernel(**inputs):
    inputs = {k: np.asarray(v, dtype=np.float32) for k, v in inputs.items()}
    task_full, _ = run_on_device(inputs, mm_dtype="f32r")
    out = np.broadcast_to(task_full[:, None, :], (B, OUT_L, D))
    return np.ascontiguousarray(out)


# revision 8
# speedup vs baseline: 1.2785x; 1.2785x over previous
"""Trainium2 Bass kernel for nn_MultiScaleFusionLayer (moe_routing).

Mathematical structure of the reference (see problem statement):
  - Every stage (expert matmuls, gate logits, mixture combine, attention
    softmax) is position-wise along L.
  - The final output is `task` (computed from gates at l=0 and attention
    scores at l=0..3) broadcast along L=100.
  => Only input positions l in {0,1,2,3} ever influence the output.

Strategy:
  - Host slices inputs to l<4 and shards batch B=2048 across 8 cores
    (256 rows/core, 4 positions => N=1024 "rows" per core, l-major).
  - Activations are shipped feature-on-partition (FT layout) with a ones
    row appended, so every bias folds into the matmuls and matmul lhsT
    (stationary operand) never needs on-device transposition.
  - One fused weight matrix computes, per 128-row tile: shared experts E
    (256 cols), the id/img/txt-feature parts of all task experts
    (3 x 256 cols), and all 16 gate logits in a single pair of
    K-accumulated matmuls. The gate_share-dependent part of the task
    experts is added with one PE transpose + 3 accumulating matmuls.
  - Expert mixture (per-row gating) runs on DVE with broadcast access
    patterns; attention runs as 2 PE transposes + matmul + softmax
    (no max subtraction needed: logits are tiny).
  - Device returns task [256, 64] per core; host broadcasts to L=100.

Weight/psum column layout uses d-major expert blocks (col = d*4 + e) so
the e-reduction is an innermost-axis DVE reduce, and combined/att use
col = d*4 + g so the final task combine is a flat elementwise mul+reduce.
"""

import sys

if "/opt/trn_rl_repo" not in sys.path:
    sys.path.insert(0, "/opt/trn_rl_repo")

import numpy as np

B, L, D = 2048, 100, 64
OUT_L = 100
NCORES = 8
BC = B // NCORES          # 256 batch rows per core
LK = 4                    # positions that matter
N = BC * LK               # 1024 rows per core (n = l*BC + b)
NT = N // 128             # 8 row-tiles of 128 rows
KA, KB = 128, 65          # xin partition split (192 features + ones)
WC = 1040                 # fused weight cols: E 256 | T1f 256 | T2f 256 | T3f 256 | sels 16

_f32 = np.float32


def pack_weights(Ws, bs, Wt1, bt1, Wt2, bt2, Wt3, bt3,
                 Wgs, bgs, Wg1, bg1, Wg2, bg2, Wg3, bg3, Watt, batt):
    """Build the fused device weight arrays (all float32, replicated per core)."""
    def blk(Wt):  # [E,Din,D] -> [Din, D*4] with col = d*4+e
        return np.ascontiguousarray(np.transpose(Wt, (1, 2, 0)).reshape(Wt.shape[1], -1))

    def bblk(bt):  # [E,D] -> [D*4]
        return np.ascontiguousarray(bt.T.reshape(-1))

    Wbig = np.zeros((193, WC), dtype=_f32)
    Wbig[0:192, 0:256] = blk(Ws)
    Wbig[192, 0:256] = bblk(bs)
    Wbig[0:64, 256:512] = blk(Wt1)
    Wbig[192, 256:512] = bblk(bt1)
    Wbig[64:128, 512:768] = blk(Wt2)
    Wbig[192, 512:768] = bblk(bt2)
    Wbig[128:192, 768:1024] = blk(Wt3)
    Wbig[192, 768:1024] = bblk(bt3)
    Wbig[0:192, 1024:1028] = Wgs
    Wbig[192, 1024:1028] = bgs
    Wbig[0:64, 1028:1032] = Wg1
    Wbig[192, 1028:1032] = bg1
    Wbig[64:128, 1032:1036] = Wg2
    Wbig[192, 1032:1036] = bg2
    Wbig[128:192, 1036:1040] = Wg3
    Wbig[192, 1036:1040] = bg3

    wgs2 = np.concatenate([blk(Wt1), blk(Wt2), blk(Wt3)], axis=1)  # [64, 768]

    # Watt rows permuted to match combined's col = d*4+g layout.
    wap = np.ascontiguousarray(
        Watt.reshape(4, 64, 64).transpose(1, 0, 2).reshape(256, 64))
    watt2 = np.concatenate([wap[0:128], wap[128:256]], axis=1)  # [128, 128]

    battb = np.ascontiguousarray(np.tile(batt[None, :], (128, 1)))  # [128, 64]
    ident = np.eye(128, dtype=_f32)

    return {
        "wbig_a": np.ascontiguousarray(Wbig[0:128]),
        "wbig_b": np.ascontiguousarray(Wbig[128:193]),
        "wgs2": np.ascontiguousarray(wgs2.astype(_f32)),
        "watt2": np.ascontiguousarray(watt2.astype(_f32)),
        "battb": battb.astype(_f32),
        "ident": ident,
    }


def pack_core_inputs(id_feat, img_feat, txt_feat, core):
    """FT-layout per-core activations: xin_a [128, N], xin_b [65, N]."""
    sl = slice(core * BC, (core + 1) * BC)

    def ft(x):  # [BC, LK, D] -> [D, N] with n = l*BC + b
        return np.ascontiguousarray(x[sl, 0:LK, :].transpose(2, 1, 0).reshape(D, N))

    fid, fimg, ftxt = ft(id_feat), ft(img_feat), ft(txt_feat)
    xin_a = np.concatenate([fid, fimg], axis=0)
    xin_b = np.concatenate([ftxt, np.ones((1, N), dtype=_f32)], axis=0)
    return xin_a, xin_b


def numpy_forward(xin_a, xin_b, w):
    """Pure-numpy mirror of the device kernel (for validating the packing)."""
    xin = np.concatenate([xin_a, xin_b], axis=0)          # [193, N]
    Wbig = np.concatenate([w["wbig_a"], w["wbig_b"]], axis=0)
    P = xin.T @ Wbig                                      # [N, 1040]
    sel = P[:, 1024:1040]                                 # [N, 16]
    task = np.zeros((BC, D), dtype=_f32)
    att_all = np.zeros((2, 128, D, LK), dtype=_f32)
    comb_keep = np.zeros((2, 128, 256), dtype=_f32)
    wap = np.concatenate([w["watt2"][:, 0:64], w["watt2"][:, 64:128]], axis=0)
    for t in range(NT):
        rows = slice(t * 128, (t + 1) * 128)
        l, j = t // 2, t % 2
        E = P[rows, 0:256].reshape(128, D, 4)
        gs = (E * sel[rows, 0:4].reshape(128, 1, 4)).sum(-1)      # [128, 64]
        Tg = P[rows, 256:1024].reshape(128, 3, D, 4).copy()
        Tg += (gs @ w["wgs2"]).reshape(128, 3, D, 4)
        combined = np.zeros((128, D, 4), dtype=_f32)
        for g in range(3):
            combined[:, :, g] = (
                Tg[:, g] * sel[rows, 4 * (g + 1):4 * (g + 2)].reshape(128, 1, 4)
            ).sum(-1)
        combined[:, :, 3] = gs
        cflat = combined.reshape(128, 256)
        logits = cflat @ wap + w["battb"][0]
        ex = np.exp(logits)
        att = ex / ex.sum(-1, keepdims=True)
        att_all[j, :, :, l] = att
        if l == 0:
            comb_keep[j] = cflat
    for j in range(2):
        prod = comb_keep[j] * att_all[j].reshape(128, 256)
        task[j * 128:(j + 1) * 128] = prod.reshape(128, D, 4).sum(-1)
    return task


# ---------------------------------------------------------------------------
# Bass program
# ---------------------------------------------------------------------------

def build_program(mm_dtype="f32r"):
    """Build the Bass/Tile program. Returns (nc, out_name)."""
    import concourse.bass as bass
    import concourse.bacc as bacc
    import concourse.mybir as mybir
    import concourse.tile as tile
    from contextlib import ExitStack

    f32 = mybir.dt.float32
    mmdt = {
        "f32": mybir.dt.float32,
        "f32r": mybir.dt.float32r,
    }[mm_dtype]

    nc = bacc.Bacc("TRN2", target_bir_lowering=False, debug=False)

    xin_a = nc.dram_tensor("xin_a", [KA, N], f32, kind="ExternalInput").ap()
    xin_b = nc.dram_tensor("xin_b", [KB, N], f32, kind="ExternalInput").ap()
    wbig_a = nc.dram_tensor("wbig_a", [KA, WC], f32, kind="ExternalInput").ap()
    wbig_b = nc.dram_tensor("wbig_b", [KB, WC], f32, kind="ExternalInput").ap()
    wgs2 = nc.dram_tensor("wgs2", [64, 768], f32, kind="ExternalInput").ap()
    watt2 = nc.dram_tensor("watt2", [128, 128], f32, kind="ExternalInput").ap()
    battb = nc.dram_tensor("battb", [128, 64], f32, kind="ExternalInput").ap()
    ident = nc.dram_tensor("ident", [128, 128], f32, kind="ExternalInput").ap()
    out = nc.dram_tensor("task", [BC, D], f32, kind="ExternalOutput").ap()

    Exp = mybir.ActivationFunctionType.Exp
    mult = mybir.AluOpType.mult
    add_op = mybir.AluOpType.add
    AX = mybir.AxisListType.X
    PSUM = bass.MemorySpace.PSUM

    with tile.TileContext(nc) as tc, ExitStack() as ctx:
        wp = ctx.enter_context(tc.tile_pool(name="w", bufs=1))
        xp = ctx.enter_context(tc.tile_pool(name="x", bufs=1))
        work = ctx.enter_context(tc.tile_pool(name="work", bufs=3))
        keep = ctx.enter_context(tc.tile_pool(name="keep", bufs=1))
        pbe_pool = ctx.enter_context(tc.tile_pool(name="pbe", bufs=1, space=PSUM))
        pbt_pool = ctx.enter_context(tc.tile_pool(name="pbt", bufs=2, space=PSUM))
        ps_small = ctx.enter_context(tc.tile_pool(name="pssm", bufs=1, space=PSUM))

        # --- load weights + activations (weights first: they gate the MMs)
        wa = wp.tile([KA, WC], f32, tag="wa")
        wb = wp.tile([KB, WC], f32, tag="wb")
        wg = wp.tile([64, 768], f32, tag="wg")
        wt = wp.tile([128, 128], f32, tag="wt")
        bb = wp.tile([128, 64], f32, tag="bb")
        idn = wp.tile([128, 128], f32, tag="idn")
        for dst, src in ((wa, wbig_a), (wb, wbig_b), (wg, wgs2),
                         (wt, watt2), (bb, battb), (idn, ident)):
            nc.sync.dma_start(dst[:], src[:])

        xa = xp.tile([KA, N], f32, tag="xa")
        xb = xp.tile([KB, N], f32, tag="xb")
        nc.sync.dma_start(xa[:], xin_a[:])
        nc.sync.dma_start(xb[:], xin_b[:])

        if mmdt is f32:
            xar, xbr, war, wbr, wgr, wtr = xa, xb, wa, wb, wg, wt
        else:
            # fp32r matmul operands must come from an instruction that
            # rounds to fp32r precision; DMA cannot, so cast-copy once.
            xar = wp.tile([KA, N], mmdt, tag="xar")
            xbr = wp.tile([KB, N], mmdt, tag="xbr")
            war = wp.tile([KA, WC], mmdt, tag="war")
            wbr = wp.tile([KB, WC], mmdt, tag="wbr")
            wgr = wp.tile([64, 768], mmdt, tag="wgr")
            wtr = wp.tile([128, 128], mmdt, tag="wtr")
            nc.scalar.copy(war[:], wa[:])
            nc.scalar.copy(wbr[:], wb[:])
            nc.vector.tensor_copy(wgr[:], wg[:])
            nc.vector.tensor_copy(wtr[:], wt[:])
            nc.vector.tensor_copy(xar[:], xa[:])
            nc.vector.tensor_copy(xbr[:], xb[:])

        # persistent accumulators for the final combine
        att_all = [keep.tile([128, D, LK], f32, tag=f"att{j}", name=f"att{j}")
                   for j in range(2)]
        comb_keep = [keep.tile([128, 256], f32, tag=f"ck{j}", name=f"ck{j}")
                     for j in range(2)]

        for t in range(NT):
            l, j = t // 2, t % 2
            cols = bass.ts(t, 128)

            # pb_e: one PSUM bank holding E [0:256] + gate logits [256:272]
            # pb_t: two banks: T1f [0:256] T2f [256:512] | T3f [512:768]
            # pbca: one bank: combined^T [0:256] + att logits [256:320]
            # One matmul accumulation group per bank: exactly one start=True
            # (it lazily zeroes the whole 2 KiB bank) and one stop=True.
            pb_e = pbe_pool.tile([128, 272], f32, tag="pbe")
            pb_t = pbt_pool.tile([128, 768], f32, tag="pbt")
            gst_ps = ps_small.tile([64, 128], f32, tag="gst")
            ct_ps = ps_small.tile([128, 256], f32, tag="ctps")
            att_ps = ps_small.tile([128, 64], f32, tag="attps")

            nc.tensor.matmul(pb_e[:, 0:256], xar[:, cols],
                             war[:, 0:256], start=True, stop=False)
            nc.tensor.matmul(pb_e[:, 256:272], xar[:, cols],
                             war[:, 1024:WC], start=False, stop=False)
            nc.tensor.matmul(pb_e[:, 0:256], xbr[:, cols],
                             wbr[:, 0:256], start=False, stop=False)
            nc.tensor.matmul(pb_e[:, 256:272], xbr[:, cols],
                             wbr[:, 1024:WC], start=False, stop=True)

            nc.tensor.matmul(pb_t[:, 0:256], xar[:, cols],
                             war[:, 256:512], start=True, stop=False)
            nc.tensor.matmul(pb_t[:, 256:512], xar[:, cols],
                             war[:, 512:768], start=False, stop=False)
            nc.tensor.matmul(pb_t[:, 512:768], xar[:, cols],
                             war[:, 768:1024], start=True, stop=False)
            nc.tensor.matmul(pb_t[:, 0:256], xbr[:, cols],
                             wbr[:, 256:512], start=False, stop=False)
            nc.tensor.matmul(pb_t[:, 256:512], xbr[:, cols],
                             wbr[:, 512:768], start=False, stop=False)
            nc.tensor.matmul(pb_t[:, 512:768], xbr[:, cols],
                             wbr[:, 768:1024], start=False, stop=False)

            sel = work.tile([128, 16], f32, tag="sel")
            nc.scalar.copy(sel[:], pb_e[:, 256:272])

            # gate_share = sum_e E[:, d, e] * sel_s[:, e]
            prod_s = work.tile([128, D, 4], f32, tag="prods")
            gs = work.tile([128, D], f32, tag="gs")
            sel_s_b = sel[:, 0:4].unsqueeze(1).broadcast_to((128, D, 4))
            nc.vector.tensor_tensor(prod_s[:], pb_e[:, 0:256].rearrange(
                "p (d e) -> p d e", e=4), sel_s_b, op=mult)
            nc.vector.reduce_sum(gs[:], prod_s[:], axis=AX)

            combined = comb_keep[j] if l == 0 else work.tile(
                [128, 256], f32, tag="comb")
            nc.vector.tensor_copy(combined[:, 3:256:4], gs[:])

            # transpose gate_share, add its expert contribution on PE
            nc.tensor.transpose(gst_ps[:], gs[:], idn[:])
            gst = work.tile([64, 128], mmdt, tag="gstsb")
            nc.scalar.copy(gst[:], gst_ps[:])
            nc.tensor.matmul(pb_t[:, 0:256], gst[:], wgr[:, 0:256],
                             start=False, stop=False)
            nc.tensor.matmul(pb_t[:, 256:512], gst[:], wgr[:, 256:512],
                             start=False, stop=True)
            nc.tensor.matmul(pb_t[:, 512:768], gst[:], wgr[:, 512:768],
                             start=False, stop=True)

            # task gates: g_k = sum_e T_k[:, d, e] * sel_k[:, e]
            for g in range(3):
                prod_t = work.tile([128, D, 4], f32, tag=f"prodt{g}",
                                   name=f"prodt{g}")
                sel_b = sel[:, 4 * (g + 1):4 * (g + 2)].unsqueeze(
                    1).broadcast_to((128, D, 4))
                nc.vector.tensor_tensor(
                    prod_t[:], pb_t[:, 256 * g:256 * (g + 1)].rearrange(
                        "p (d e) -> p d e", e=4), sel_b, op=mult)
                nc.vector.reduce_sum(combined[:, g:256:4], prod_t[:], axis=AX)

            # attention: transpose combined, matmul with Watt, softmax
            nc.tensor.matmul(ct_ps[:, 0:128], combined[:, 0:128], idn[:],
                             is_transpose=True, start=True, stop=False)
            nc.tensor.matmul(ct_ps[:, 128:256], combined[:, 128:256], idn[:],
                             is_transpose=True, start=False, stop=True)
            ct = work.tile([128, 256], mmdt, tag="ct")
            nc.scalar.copy(ct[:, 0:128], ct_ps[:, 0:128])
            nc.scalar.copy(ct[:, 128:256], ct_ps[:, 128:256])
            nc.tensor.matmul(att_ps[:], ct[:, 0:128],
                             wtr[:, 0:64], start=True, stop=False)
            nc.tensor.matmul(att_ps[:], ct[:, 128:256],
                             wtr[:, 64:128], start=False, stop=True)

            logit = work.tile([128, D], f32, tag="logit")
            nc.vector.tensor_tensor(logit[:], att_ps[:], bb[:],
                                    op=add_op)
            ex = work.tile([128, D], f32, tag="ex")
            sumex = work.tile([128, 1], f32, tag="sumex")
            nc.scalar.activation(ex[:], logit[:], Exp, accum_out=sumex[:])
            rinv = work.tile([128, 1], f32, tag="rinv")
            nc.vector.reciprocal(rinv[:], sumex[:])
            nc.vector.tensor_scalar_mul(
                att_all[j][:, :, l], ex[:], rinv[:])

        # final combine + store
        for j in range(2):
            prod_f = work.tile([128, D, 4], f32, tag="prodf")
            task = work.tile([128, D], f32, tag="task")
            nc.vector.tensor_tensor(
                prod_f[:], comb_keep[j][:].rearrange("p (d g) -> p d g", g=4),
                att_all[j][:], op=mult)
            nc.vector.reduce_sum(task[:], prod_f[:], axis=AX)
            nc.sync.dma_start(out[j * 128:(j + 1) * 128, :], task[:])

    nc.compile()
    return nc, "task"


_PROGRAM_CACHE = {}


def _get_program(mm_dtype):
    if mm_dtype not in _PROGRAM_CACHE:
        _PROGRAM_CACHE[mm_dtype] = build_program(mm_dtype)
    return _PROGRAM_CACHE[mm_dtype]


def run_on_device(inputs, mm_dtype="f32r", trace=False):
    """Shard, run on 8 cores, return (task_full [B, D], BassKernelResults)."""
    from concourse.bass_utils import run_bass_kernel_spmd

    nc, out_name = _get_program(mm_dtype)
    w = pack_weights(**{k: inputs[k] for k in (
        "Ws", "bs", "Wt1", "bt1", "Wt2", "bt2", "Wt3", "bt3",
        "Wgs", "bgs", "Wg1", "bg1", "Wg2", "bg2", "Wg3", "bg3",
        "Watt", "batt")})
    in_maps = []
    for c in range(NCORES):
        xin_a, xin_b = pack_core_inputs(
            inputs["id_feat"], inputs["img_feat"], inputs["txt_feat"], c)
        in_maps.append({"xin_a": xin_a, "xin_b": xin_b, **w})
    res = run_bass_kernel_spmd(nc, in_maps, core_ids=list(range(NCORES)),
                               trace=trace)
    task_full = np.concatenate(
        [res.results[c][out_name] for c in range(NCORES)], axis=0)
    return task_full, res


def kernel(**inputs):
    inputs = {k: np.asarray(v, dtype=np.float32) for k, v in inputs.items()}
    task_full, _ = run_on_device(inputs, mm_dtype="f32r")
    out = np.broadcast_to(task_full[:, None, :], (B, OUT_L, D))
    return np.ascontiguousarray(out)
